# revision 20
# baseline (speedup 1.0000x reference)
"""DSS layer (LN -> long causal conv via overlap-save DFT matmuls -> +residual)
on 8 axon-tunneled TRN2 NeuronCores, written in Bass/Tile.

Wall-clock on this setup is dominated by the shared ~45 MB/s axon tunnel
(~85 ms round-trip latency; the CPU is only ~35% busy while the wire streams,
so host numpy CAN hide inside transfer windows, but separate device_put calls
and extra synchronous round trips are expensive). The design minimizes
transferred bytes and keeps the wire busy end-to-end:
  sharding: one program where core = (batch in a pair) x (channel quarter);
        dispatched twice (batches 0-1, then 2-3). Each core owns the full
        sequence for 256 channels -> causal conv needs NO halo rows (total
        upload exactly L*D int8 = 16.8 MB). The split pipelines the tunnel:
        quant of pair B runs in the idle CPU while pair A uploads, exec A's
        round trip hides under pair B's upload stream, dequant of pair A
        hides under pair B's download stream, and exec B hides under pair
        A's download. Both fetches run in threads so their requests are
        queued before any host work.
  host:  per-row mean/var of x (reductions only, no centered temp), per
        (row, quarter) amax of RAW x, int8 quantize q = round(x * 127/amax)
        via the 1.5*2^23 magic-add trick (no rint pass). The LN affine is
        folded into per-row scale a = amax*rstd/127 and offset o = -mu*rstd
        applied on device: u = q*a + o (fused tensor_scalar mul+add). A
        helper thread pre-faults the 64 MB output array during the wire wait.
  device (per core): dequant -> windowed rFFT-as-matmul (shared F, 8 x
        1024-row overlap-save windows, first window zero history), pointwise
        *Kf (gamma / D-residual delta tap / per-channel output scale folded
        in), inverse rFFT-as-matmul (shared G) -> centered int8 quantize.
  download int8, host dequant = single multiply by s_d; beta offset exact
        (off = beta * (cumsum(K) + D) — NOT gamma-scaled: the reference
        convolves raw K against u*gamma + beta).

Transfers ride the jitted shard_map dispatches; constants and the Kf spectrum
are cached on device across calls.
"""
import hashlib
import threading
import numpy as np
import ml_dtypes

B, L, D, N = 4, 4096, 1024, 512
NQ = 4              # channel quarters
DH = D // NQ        # 256 channels per core
CH = 512            # output chunk per window
M = 1024            # DFT window (overlap-save)
KT = 513            # kernel taps kept (<= M - CH + 1): exact for decaying K
KF = M // 2 + 1     # 513 rfft bins
NW = L // CH        # 8 windows per core
NCORE = 8
BPC = 2             # batches per call
LN_EPS = 1e-5
QCLIP = 5.2
KPART = [(0, 128), (128, 128), (256, 128), (384, 128), (512, 1)]

_S = {}
_LOCK = threading.Lock()


# ---------------------------------------------------------------- device kernel
def _build_nc():
    import concourse.bacc as bacc
    import concourse.mybir as mybir
    import concourse.tile as tile

    dt = mybir.dt
    nc = bacc.Bacc("TRN2", target_bir_lowering=False, debug=False, num_devices=NCORE)
    uq_d = nc.dram_tensor("uq", [L, DH], dt.int8, kind="ExternalInput").ap()
    sab_d = nc.dram_tensor("sab", [2 * L], dt.float32, kind="ExternalInput").ap()
    kr_d = nc.dram_tensor("kr", [KF, DH], dt.bfloat16, kind="ExternalInput").ap()
    ki_d = nc.dram_tensor("ki", [KF, DH], dt.bfloat16, kind="ExternalInput").ap()
    fc_d = nc.dram_tensor("fc", [M, KF], dt.bfloat16, kind="ExternalInput").ap()
    fs_d = nc.dram_tensor("fs", [M, KF], dt.bfloat16, kind="ExternalInput").ap()
    gr_d = nc.dram_tensor("gr", [KF, CH], dt.bfloat16, kind="ExternalInput").ap()
    gi_d = nc.dram_tensor("gi", [KF, CH], dt.bfloat16, kind="ExternalInput").ap()
    yq_d = nc.dram_tensor("yq", [L, DH], dt.int8, kind="ExternalOutput").ap()

    nT = L // 128  # 32 row tiles of 128

    with tile.TileContext(nc) as tc:
        with (
            tc.tile_pool(name="const", bufs=1) as constp,
            tc.tile_pool(name="stage", bufs=2) as stagep,
            tc.tile_pool(name="upool", bufs=9) as upool,
            tc.tile_pool(name="uv", bufs=2) as uvp,
            tc.tile_pool(name="work", bufs=2) as workp,
            tc.tile_pool(name="psum", bufs=4, space="PSUM") as psump,
            tc.tile_pool(name="psumi", bufs=2, space="PSUM") as psumip,
        ):
            def widen(dram_ap, rows, cols, tagn):
                st = stagep.tile([rows, cols], dt.bfloat16, tag="stage")
                nc.sync.dma_start(st[:], dram_ap)
                ft = constp.tile([rows, cols], dt.float32, tag=tagn)
                nc.vector.tensor_copy(ft[:], st[:])
                return ft

            fc_t = [widen(fc_d[i * 128:(i + 1) * 128, :], 128, KF, f"fc{i}") for i in range(8)]
            fs_t = [widen(fs_d[i * 128:(i + 1) * 128, :], 128, KF, f"fs{i}") for i in range(8)]
            gr_t = [widen(gr_d[o:o + w, :], w, CH, f"gr{i}") for i, (o, w) in enumerate(KPART)]
            gi_t = [widen(gi_d[o:o + w, :], w, CH, f"gi{i}") for i, (o, w) in enumerate(KPART)]

            # Kf stays bf16 in SBUF (read by DVE pointwise; halves footprint)
            def load_bf(dram_ap, rows, cols, tagn):
                t = constp.tile([rows, cols], dt.bfloat16, tag=tagn)
                nc.sync.dma_start(t[:], dram_ap)
                return t

            kr_t = [load_bf(kr_d[o:o + w, :], w, DH, f"kr{i}") for i, (o, w) in enumerate(KPART)]
            ki_t = [load_bf(ki_d[o:o + w, :], w, DH, f"ki{i}") for i, (o, w) in enumerate(KPART)]

            sa_raw = constp.tile([128, nT], dt.float32, tag="sa_raw")
            nc.sync.dma_start(sa_raw[:], sab_d[0:L].rearrange("(n p) -> p n", p=128))
            so_raw = constp.tile([128, nT], dt.float32, tag="so_raw")
            nc.sync.dma_start(so_raw[:], sab_d[L:2 * L].rearrange("(n p) -> p n", p=128))
            # staged via same-engine copy so dequant TensorScalarPtr needs no waits
            sa_sb = constp.tile([128, nT], dt.float32, tag="sa_sb")
            nc.vector.tensor_copy(sa_sb[:], sa_raw[:])
            so_sb = constp.tile([128, nT], dt.float32, tag="so_sb")
            nc.vector.tensor_copy(so_sb[:], so_raw[:])

            mult = mybir.AluOpType.mult
            add = mybir.AluOpType.add

            for w in range(NW):
                # window rows [w*512-512, w*512+512); slot si holds global tile
                # gt = w*4 - 4 + si; for w == 0 slots 0-3 are zero history and
                # their matmuls are skipped entirely.
                s0 = 4 if w == 0 else 0
                u_t = [None] * 8
                for si in range(s0, 8):
                    gt = w * 4 - 4 + si
                    stq = stagep.tile([128, DH], dt.int8, tag="uqstage")
                    nc.sync.dma_start(stq[:], uq_d[gt * 128:(gt + 1) * 128, :])
                    uf = upool.tile([128, DH], dt.float32, tag="u")
                    nc.vector.tensor_scalar(
                        uf[:], stq[:], sa_sb[:, gt:gt + 1], so_sb[:, gt:gt + 1],
                        mult, add,
                    )
                    u_t[si] = uf
                Vr, Vi = [], []
                for it, (ko, kw) in enumerate(KPART):
                    sb_ri = []
                    for nm, fT in (("ur", fc_t), ("ui", fs_t)):
                        ps = psump.tile([kw, DH], dt.float32, tag="psf")
                        for si in range(s0, 8):
                            nc.tensor.matmul(
                                ps[:], fT[si][:, ko:ko + kw], u_t[si][:],
                                start=(si == s0), stop=(si == 7),
                            )
                        sb = uvp.tile([kw, DH], dt.float32, tag=nm)
                        nc.scalar.copy(sb[:], ps[:])
                        sb_ri.append(sb)
                    ur, ui = sb_ri
                    krs, kis = kr_t[it][:kw, :], ki_t[it][:kw, :]
                    t1 = workp.tile([kw, DH], dt.float32, tag="t1")
                    t2 = workp.tile([kw, DH], dt.float32, tag="t2")
                    nc.vector.tensor_mul(t1[:], ur[:], krs)
                    nc.vector.tensor_mul(t2[:], ui[:], kis)
                    vr = uvp.tile([kw, DH], dt.float32, tag=f"vr{it}")
                    nc.vector.tensor_sub(vr[:], t1[:], t2[:])
                    t3 = workp.tile([kw, DH], dt.float32, tag="t3")
                    t4 = workp.tile([kw, DH], dt.float32, tag="t4")
                    nc.vector.tensor_mul(t3[:], ur[:], kis)
                    nc.vector.tensor_mul(t4[:], ui[:], krs)
                    vi = uvp.tile([kw, DH], dt.float32, tag=f"vi{it}")
                    nc.vector.tensor_add(vi[:], t3[:], t4[:])
                    Vr.append(vr)
                    Vi.append(vi)
                for tt in range(4):
                    ps = psumip.tile([128, DH], dt.float32, tag="psi")
                    mm = 0
                    for gT, V in ((gr_t, Vr), (gi_t, Vi)):
                        for it, (ko, kw) in enumerate(KPART):
                            nc.tensor.matmul(
                                ps[:], gT[it][:kw, tt * 128:(tt + 1) * 128], V[it][:],
                                start=(mm == 0), stop=(mm == 9),
                            )
                            mm += 1
                    yf = workp.tile([128, DH], dt.float32, tag="yf")
                    nc.vector.tensor_scalar(yf[:], ps[:], -127.0, 127.0,
                                            mybir.AluOpType.max,
                                            mybir.AluOpType.min)
                    yq_t = workp.tile([128, DH], dt.int8, tag="yqt")
                    nc.vector.tensor_copy(yq_t[:], yf[:])
                    nc.sync.dma_start(
                        yq_d[w * CH + tt * 128: w * CH + (tt + 1) * 128, :], yq_t[:]
                    )
    nc.finalize()
    return nc


# ---------------------------------------------------------------- runner
def _make_runner(nc):
    import jax
    from jax.sharding import Mesh, PartitionSpec
    from jax.experimental.shard_map import shard_map
    import concourse.mybir as mybir
    from concourse.bass2jax import install_neuronx_cc_hook, _bass_exec_p, partition_id_tensor

    install_neuronx_cc_hook()
    in_names, out_names, out_avals, zero_outs = [], [], [], []
    partition_name = nc.partition_id_tensor.name if nc.partition_id_tensor else None
    for alloc in nc.m.functions[0].allocations:
        if not isinstance(alloc, mybir.MemoryLocationSet):
            continue
        name = alloc.memorylocations[0].name
        if alloc.kind == "ExternalInput":
            if name != partition_name:
                in_names.append(name)
        elif alloc.kind == "ExternalOutput":
            out_names.append(name)
            shape = tuple(alloc.tensor_shape)
            dtype = mybir.dt.np(alloc.dtype)
            out_avals.append(jax.core.ShapedArray(shape, dtype))
            zero_outs.append(np.zeros(shape, dtype))
    n_params = len(in_names)
    all_names = in_names + out_names
    if partition_name is not None:
        all_names.append(partition_name)

    def _body(*args):
        operands = list(args)
        if partition_name is not None:
            operands.append(partition_id_tensor())
        outs = _bass_exec_p.bind(
            *operands,
            out_avals=tuple(out_avals),
            in_names=tuple(all_names),
            out_names=tuple(out_names),
            lowering_input_output_aliases=(),
            sim_require_finite=True,
            sim_require_nnan=True,
            nc=nc,
        )
        return tuple(outs)

    devices = jax.devices()[:NCORE]
    mesh = Mesh(np.asarray(devices), ("core",))
    n_outs = len(out_names)
    sharded = jax.jit(
        shard_map(
            _body, mesh=mesh,
            in_specs=(PartitionSpec("core"),) * (n_params + n_outs),
            out_specs=(PartitionSpec("core"),) * n_outs,
            check_rep=False,
        ),
        keep_unused=True,
    )
    return sharded, in_names, out_names, zero_outs, mesh


def _dft_consts():
    t = np.arange(M)
    k = np.arange(KF)
    ang = 2.0 * np.pi / M * np.outer(t, k)
    fc = np.cos(ang)
    fs = -np.sin(ang)
    w_k = np.where((k == 0) | (k == M // 2), 1.0, 2.0) / M
    angi = 2.0 * np.pi / M * np.outer(k, np.arange(CH, M))
    gr = w_k[:, None] * np.cos(angi)
    gi = -w_k[:, None] * np.sin(angi)
    bf = ml_dtypes.bfloat16
    return (fc.astype(bf), fs.astype(bf), gr.astype(bf), gi.astype(bf))


def _init():
    import jax
    from jax.sharding import NamedSharding, PartitionSpec

    nc = _build_nc()
    sharded, in_names, out_names, zero_outs, mesh = _make_runner(nc)
    assert in_names == ["uq", "sab", "kr", "ki", "fc", "fs", "gr", "gi"], in_names
    repl = NamedSharding(mesh, PartitionSpec("core"))
    fc, fs, gr, gi = _dft_consts()
    tile8 = lambda a: jax.device_put(np.tile(a, (NCORE, 1)), repl)
    _S["fgdev"] = (tile8(fc), tile8(fs), tile8(gr), tile8(gi))
    zc = np.zeros((NCORE * L, DH), np.int8)
    _S["zeros"] = jax.device_put(zc, repl)
    _S["sharded"] = sharded
    _S["repl"] = repl
    _S["kcache"] = {}
    try:
        _S["cq"] = _build_cquant()
    except Exception:
        _S["cq"] = None  # numpy fallback
    _S["ready"] = True


# ---------------------------------------------------------------- host math
def _host_precompute(Lr, Li, Cr, Ci, Dp, g, b):
    lam = -np.exp(Lr.astype(np.float64)) + 1j * np.exp(Li.astype(np.float64))
    Ct = (Cr.astype(np.float64) + 1j * Ci.astype(np.float64)) * (np.exp(lam) - 1.0) / lam
    tau = np.arange(KT)
    E = np.exp(lam[None, :] * tau[:, None])  # [KT, N]
    K = (E.real.astype(np.float32) @ Ct.real.T.astype(np.float32)
         - E.imag.astype(np.float32) @ Ct.imag.T.astype(np.float32))  # [KT, D]
    gf = g.astype(np.float32)
    Khat = K * gf[None, :]
    Khat[0] += (Dp * g).astype(np.float32)
    sigma = np.sqrt((Khat.astype(np.float64) ** 2).sum(0))
    s_d = np.maximum(QCLIP * sigma / 127.0, 1e-12).astype(np.float32)
    Kf = np.fft.rfft(Khat / s_d[None, :], n=M, axis=0)
    bf = ml_dtypes.bfloat16
    kr = Kf.real.astype(bf)
    ki = Kf.imag.astype(bf)
    # per-core d-quarter slices, stacked core-major for the sharded input
    krs = np.ascontiguousarray(
        np.concatenate([kr[:, (c % NQ) * DH:(c % NQ + 1) * DH] for c in range(NCORE)], 0))
    kis = np.ascontiguousarray(
        np.concatenate([ki[:, (c % NQ) * DH:(c % NQ + 1) * DH] for c in range(NCORE)], 0))
    if np.any(b):
        # beta rides through the raw conv + residual: conv(beta*1, K) + D*beta
        csK = np.cumsum(K, axis=0)
        off = b.astype(np.float32)[None, :] * (csK + Dp.astype(np.float32)[None, :])
    else:
        off = None
    return krs, kis, s_d, off


_C_SRC = r'''
#include <stdint.h>
#include <math.h>
#define L 4096
#define D 1024
#define NQ 4
#define DH 256
/* Fused LN-stats + raw-x int8 quantization for one batch pair.
   Per row: one cached read computes sum/sumsq + per-quarter max/min, then
   quantizes from L1. Matches the numpy path: q = round(x * 127/amax),
   sab[c][0][l] = amax*rstd/127, sab[c][1][l] = -mu*rstd. */
void quant_pair(const float* restrict xp, int8_t* restrict uq,
                float* restrict sab) {
    for (int bl = 0; bl < 2; bl++) {
        const float* xb = xp + (long)bl * L * D;
        for (int l = 0; l < L; l++) {
            const float* row = xb + (long)l * D;
            double s = 0.0, ss = 0.0;
            float mx[NQ], mn[NQ];
            for (int q = 0; q < NQ; q++) {
                const float* seg = row + q * DH;
                float qmx = seg[0], qmn = seg[0], qs = 0.0f, qss = 0.0f;
                for (int j = 0; j < DH; j++) {
                    float v = seg[j];
                    qs += v; qss += v * v;
                    qmx = v > qmx ? v : qmx;
                    qmn = v < qmn ? v : qmn;
                }
                s += qs; ss += qss;
                mx[q] = qmx; mn[q] = qmn;
            }
            float mu = (float)(s / D);
            float var = (float)(ss / D) - mu * mu;
            var = fmaxf(var, 0.0f);  /* f32 cancellation can go negative on
                                        (near-)constant rows -> sqrt(NaN) */
            float rstd = 1.0f / sqrtf(var + 1e-5f);
            float nmu = -mu * rstd;
            for (int q = 0; q < NQ; q++) {
                float amax = fmaxf(mx[q], -mn[q]);
                amax = fmaxf(amax, 1e-30f);
                float sq = 127.0f / amax;
                const float* seg = row + q * DH;
                int c = NQ * bl + q;
                int8_t* out = uq + ((long)c * L + l) * DH;
                /* round-to-nearest-even via the 1.5*2^23 magic add; the low
                   byte of the float is the int8 (vectorizes, unlike lrintf) */
                for (int j = 0; j < DH; j++) {
                    union { float f; uint32_t u; } t;
                    t.f = seg[j] * sq + 12582912.0f;
                    out[j] = (int8_t)(t.u & 0xffu);
                }
                sab[((long)c * 2 + 0) * L + l] = amax * rstd * (1.0f / 127.0f);
                sab[((long)c * 2 + 1) * L + l] = nmu;
            }
        }
    }
}
'''


def _build_cquant():
    import ctypes, os, subprocess
    h = hashlib.sha1(_C_SRC.encode()).hexdigest()[:16]
    so = f"/tmp/dssq_{h}.so"
    if not os.path.exists(so):
        cf = f"/tmp/dssq_{h}.c"
        with open(cf, "w") as f:
            f.write(_C_SRC)
        subprocess.run(
            ["gcc", "-O3", "-march=native", "-funroll-loops", "-shared",
             "-fPIC", "-o", so + ".tmp", cf],
            check=True, capture_output=True)
        os.replace(so + ".tmp", so)
    lib = ctypes.CDLL(so)
    lib.quant_pair.argtypes = [ctypes.c_void_p] * 3
    lib.quant_pair.restype = None
    return lib


_MAGIC = np.float32(12582912.0)  # 1.5 * 2^23: adding forces round-to-nearest-even
                                 # into the mantissa; low byte is then the int8.


def _quant_pair(x, p, uq_cc, sab_cc, tf):
    """LN stats + raw-x int8 quantization for batch pair p (batches 2p, 2p+1).

    Core c of the call handles batch 2p + c//NQ, channel quarter c%NQ.
    sab_cc[c] holds [scale_row | offset_row] (2L floats) for core c.
    """
    tb = tf.view(np.int8)[:, ::4]  # low byte of each f32 (little-endian)
    for bl in range(BPC):
        bi = BPC * p + bl
        xb = x[bi]
        mu = xb.mean(1)
        sq = np.einsum('lc,lc->l', xb, xb, optimize=True)
        var = sq * (1.0 / D) - mu * mu
        rstd = 1.0 / np.sqrt(var + LN_EPS)
        nmu = -mu * rstd
        for q in range(NQ):
            c = NQ * bl + q
            xh = xb[:, q * DH:(q + 1) * DH]
            amax = np.maximum(xh.max(1), -xh.min(1))
            np.maximum(amax, 1e-30, out=amax)
            s_q = np.float32(127.0) / amax
            np.multiply(xh, s_q[:, None], out=tf)
            np.add(tf, _MAGIC, out=tf)
            np.copyto(uq_cc[c * L:(c + 1) * L], tb)
            sab_cc[c, 0] = amax * rstd * np.float32(1.0 / 127.0)
            sab_cc[c, 1] = nmu


def kernel(x, Lambda_real, Lambda_imag, C_real, C_imag, param_D, ln_gamma, ln_beta):
    with _LOCK:
        if not _S.get("ready"):
            _init()

    x = np.asarray(x)
    small = [np.asarray(a) for a in (Lambda_real, Lambda_imag, C_real, C_imag,
                                     param_D, ln_gamma, ln_beta)]
    key = hashlib.sha1(b"".join(a.tobytes() for a in small)).hexdigest()
    kc = _S["kcache"]
    if key not in kc:
        import jax
        krs, kis, s_d, off = _host_precompute(*small)
        kr_dev = jax.device_put(krs, _S["repl"])
        ki_dev = jax.device_put(kis, _S["repl"])
        kc.clear()
        kc[key] = (kr_dev, ki_dev, s_d, off)
    kr_dev, ki_dev, s_d, off = kc[key]
    fc_dev, fs_dev, gr_dev, gi_dev = _S["fgdev"]
    sharded, zeros = _S["sharded"], _S["zeros"]

    # Staging buffers are reused across calls (internal only — the returned y
    # is always fresh; previous call's transfers completed before its fetches
    # returned, so overwriting here is safe).
    if "bufs" not in _S:
        _S["bufs"] = (np.empty((L, DH), np.float32),
                      [np.empty((NCORE * L, DH), np.int8) for _ in range(2)],
                      [np.empty((NCORE, 2, L), np.float32) for _ in range(2)])
    tf, uq, sab = _S["bufs"]

    # quantize pair A -> dispatch A (h2d streams while we quantize pair B in
    # the ~65% idle CPU) -> dispatch B; exec A's round trip hides under B's
    # upload, exec B under A's download.
    cq = _S["cq"]
    if cq is not None and not (x.flags.c_contiguous and x.dtype == np.float32):
        x = np.ascontiguousarray(x, np.float32)
    outs = [None, None]
    for p in range(2):
        if cq is not None:
            cq.quant_pair(x.ctypes.data + x.strides[0] * BPC * p,
                          uq[p].ctypes.data, sab[p].ctypes.data)
        else:
            _quant_pair(x, p, uq[p], sab[p], tf)
        outs[p] = sharded(uq[p], sab[p].reshape(-1),
                          kr_dev, ki_dev, fc_dev, fs_dev, gr_dev, gi_dev,
                          zeros)

    # Fetch both outputs from threads so both d2h requests are queued up
    # front; pre-fault y's pages during the wire wait.
    y = np.empty((B, L, D), np.float32)
    yqs = [None, None]

    def _fetch(p):
        yqs[p] = np.asarray(outs[p][0]).reshape(NCORE, L, DH)

    ths = [threading.Thread(target=_fetch, args=(p,)) for p in range(2)]
    for t in ths:
        t.start()
    y.reshape(-1)[::1024] = 0.0  # touch every 4 KB page while the wire streams

    for p in range(2):
        ths[p].join()
        yq = yqs[p]
        for c in range(NCORE):
            bi, q = BPC * p + c // NQ, c % NQ
            sd_q = s_d[q * DH:(q + 1) * DH]
            dst = y[bi, :, q * DH:(q + 1) * DH]
            np.multiply(yq[c], sd_q[None, :], out=dst)
    if off is not None:
        y[:, :KT] += off[None]
        y[:, KT:] += off[-1][None, None]
    return y


# revision 24
# speedup vs baseline: 1.0974x; 1.0974x over previous
"""DSS layer (LN -> long causal conv via overlap-save DFT matmuls -> +residual)
on 8 axon-tunneled TRN2 NeuronCores, written in Bass/Tile.

Wall-clock on this setup is dominated by the shared ~45 MB/s axon tunnel
(~85 ms round-trip latency; the CPU is only ~35% busy while the wire streams,
so host numpy CAN hide inside transfer windows, but separate device_put calls
and extra synchronous round trips are expensive). The design minimizes
transferred bytes and keeps the wire busy end-to-end:
  sharding: one program where core = (batch in a pair) x (channel quarter);
        dispatched twice (batches 0-1, then 2-3). Each core owns the full
        sequence for 256 channels -> causal conv needs NO halo rows (total
        upload exactly L*D int8 = 16.8 MB). The split pipelines the tunnel:
        quant of pair B runs in the idle CPU while pair A uploads, exec A's
        round trip hides under pair B's upload stream, dequant of pair A
        hides under pair B's download stream, and exec B hides under pair
        A's download. Both fetches run in threads so their requests are
        queued before any host work.
  host:  per-row mean/var of x (reductions only, no centered temp), per
        (row, quarter) amax of RAW x, int8 quantize q = round(x * 127/amax)
        via the 1.5*2^23 magic-add trick (no rint pass). The LN affine is
        folded into per-row scale a = amax*rstd/127 and offset o = -mu*rstd
        applied on device: u = q*a + o (fused tensor_scalar mul+add). A
        helper thread pre-faults the 64 MB output array during the wire wait.
  device (per core): dequant -> windowed rFFT-as-matmul (shared F, 8 x
        1024-row overlap-save windows, first window zero history), pointwise
        *Kf (gamma / D-residual delta tap / per-channel output scale folded
        in), inverse rFFT-as-matmul (shared G) -> centered int8 quantize.
  download int8, host dequant = single multiply by s_d; beta offset exact
        (off = beta * (cumsum(K) + D) — NOT gamma-scaled: the reference
        convolves raw K against u*gamma + beta).

Transfers ride the jitted shard_map dispatches; constants and the Kf spectrum
are cached on device across calls.
"""
import hashlib
import threading
import numpy as np
import ml_dtypes

B, L, D, N = 4, 4096, 1024, 512
NQ = 4              # channel quarters
DH = D // NQ        # 256 channels per core
CH = 512            # output chunk per window
M = 1024            # DFT window (overlap-save)
KT = 513            # kernel taps kept (<= M - CH + 1): exact for decaying K
KF = M // 2 + 1     # 513 rfft bins
NW = L // CH        # 8 windows per core
NCORE = 8
BPC = 2             # batches per call
LN_EPS = 1e-5
QCLIP = 5.2
KPART = [(0, 128), (128, 128), (256, 128), (384, 128), (512, 1)]

_S = {}
_LOCK = threading.Lock()


# ---------------------------------------------------------------- device kernel
def _build_nc():
    import concourse.bacc as bacc
    import concourse.mybir as mybir
    import concourse.tile as tile

    dt = mybir.dt
    nc = bacc.Bacc("TRN2", target_bir_lowering=False, debug=False, num_devices=NCORE)
    uq_d = nc.dram_tensor("uq", [L, DH], dt.int8, kind="ExternalInput").ap()
    sab_d = nc.dram_tensor("sab", [2 * L], dt.float32, kind="ExternalInput").ap()
    kr_d = nc.dram_tensor("kr", [KF, DH], dt.bfloat16, kind="ExternalInput").ap()
    ki_d = nc.dram_tensor("ki", [KF, DH], dt.bfloat16, kind="ExternalInput").ap()
    fc_d = nc.dram_tensor("fc", [M, KF], dt.bfloat16, kind="ExternalInput").ap()
    fs_d = nc.dram_tensor("fs", [M, KF], dt.bfloat16, kind="ExternalInput").ap()
    gr_d = nc.dram_tensor("gr", [KF, CH], dt.bfloat16, kind="ExternalInput").ap()
    gi_d = nc.dram_tensor("gi", [KF, CH], dt.bfloat16, kind="ExternalInput").ap()
    yq_d = nc.dram_tensor("yq", [L, DH], dt.int8, kind="ExternalOutput").ap()

    nT = L // 128  # 32 row tiles of 128

    with tile.TileContext(nc) as tc:
        with (
            tc.tile_pool(name="const", bufs=1) as constp,
            tc.tile_pool(name="stage", bufs=2) as stagep,
            tc.tile_pool(name="upool", bufs=9) as upool,
            tc.tile_pool(name="uv", bufs=2) as uvp,
            tc.tile_pool(name="work", bufs=2) as workp,
            tc.tile_pool(name="psum", bufs=4, space="PSUM") as psump,
            tc.tile_pool(name="psumi", bufs=2, space="PSUM") as psumip,
        ):
            def widen(dram_ap, rows, cols, tagn):
                st = stagep.tile([rows, cols], dt.bfloat16, tag="stage")
                nc.sync.dma_start(st[:], dram_ap)
                ft = constp.tile([rows, cols], dt.float32, tag=tagn)
                nc.vector.tensor_copy(ft[:], st[:])
                return ft

            fc_t = [widen(fc_d[i * 128:(i + 1) * 128, :], 128, KF, f"fc{i}") for i in range(8)]
            fs_t = [widen(fs_d[i * 128:(i + 1) * 128, :], 128, KF, f"fs{i}") for i in range(8)]
            gr_t = [widen(gr_d[o:o + w, :], w, CH, f"gr{i}") for i, (o, w) in enumerate(KPART)]
            gi_t = [widen(gi_d[o:o + w, :], w, CH, f"gi{i}") for i, (o, w) in enumerate(KPART)]

            # Kf stays bf16 in SBUF (read by DVE pointwise; halves footprint)
            def load_bf(dram_ap, rows, cols, tagn):
                t = constp.tile([rows, cols], dt.bfloat16, tag=tagn)
                nc.sync.dma_start(t[:], dram_ap)
                return t

            kr_t = [load_bf(kr_d[o:o + w, :], w, DH, f"kr{i}") for i, (o, w) in enumerate(KPART)]
            ki_t = [load_bf(ki_d[o:o + w, :], w, DH, f"ki{i}") for i, (o, w) in enumerate(KPART)]

            sa_raw = constp.tile([128, nT], dt.float32, tag="sa_raw")
            nc.sync.dma_start(sa_raw[:], sab_d[0:L].rearrange("(n p) -> p n", p=128))
            so_raw = constp.tile([128, nT], dt.float32, tag="so_raw")
            nc.sync.dma_start(so_raw[:], sab_d[L:2 * L].rearrange("(n p) -> p n", p=128))
            # staged via same-engine copy so dequant TensorScalarPtr needs no waits
            sa_sb = constp.tile([128, nT], dt.float32, tag="sa_sb")
            nc.vector.tensor_copy(sa_sb[:], sa_raw[:])
            so_sb = constp.tile([128, nT], dt.float32, tag="so_sb")
            nc.vector.tensor_copy(so_sb[:], so_raw[:])

            mult = mybir.AluOpType.mult
            add = mybir.AluOpType.add

            for w in range(NW):
                # window rows [w*512-512, w*512+512); slot si holds global tile
                # gt = w*4 - 4 + si; for w == 0 slots 0-3 are zero history and
                # their matmuls are skipped entirely.
                s0 = 4 if w == 0 else 0
                u_t = [None] * 8
                for si in range(s0, 8):
                    gt = w * 4 - 4 + si
                    stq = stagep.tile([128, DH], dt.int8, tag="uqstage")
                    nc.sync.dma_start(stq[:], uq_d[gt * 128:(gt + 1) * 128, :])
                    uf = upool.tile([128, DH], dt.float32, tag="u")
                    nc.vector.tensor_scalar(
                        uf[:], stq[:], sa_sb[:, gt:gt + 1], so_sb[:, gt:gt + 1],
                        mult, add,
                    )
                    u_t[si] = uf
                Vr, Vi = [], []
                for it, (ko, kw) in enumerate(KPART):
                    sb_ri = []
                    for nm, fT in (("ur", fc_t), ("ui", fs_t)):
                        ps = psump.tile([kw, DH], dt.float32, tag="psf")
                        for si in range(s0, 8):
                            nc.tensor.matmul(
                                ps[:], fT[si][:, ko:ko + kw], u_t[si][:],
                                start=(si == s0), stop=(si == 7),
                            )
                        sb = uvp.tile([kw, DH], dt.float32, tag=nm)
                        nc.scalar.copy(sb[:], ps[:])
                        sb_ri.append(sb)
                    ur, ui = sb_ri
                    krs, kis = kr_t[it][:kw, :], ki_t[it][:kw, :]
                    t1 = workp.tile([kw, DH], dt.float32, tag="t1")
                    t2 = workp.tile([kw, DH], dt.float32, tag="t2")
                    nc.vector.tensor_mul(t1[:], ur[:], krs)
                    nc.vector.tensor_mul(t2[:], ui[:], kis)
                    vr = uvp.tile([kw, DH], dt.float32, tag=f"vr{it}")
                    nc.vector.tensor_sub(vr[:], t1[:], t2[:])
                    t3 = workp.tile([kw, DH], dt.float32, tag="t3")
                    t4 = workp.tile([kw, DH], dt.float32, tag="t4")
                    nc.vector.tensor_mul(t3[:], ur[:], kis)
                    nc.vector.tensor_mul(t4[:], ui[:], krs)
                    vi = uvp.tile([kw, DH], dt.float32, tag=f"vi{it}")
                    nc.vector.tensor_add(vi[:], t3[:], t4[:])
                    Vr.append(vr)
                    Vi.append(vi)
                for tt in range(4):
                    ps = psumip.tile([128, DH], dt.float32, tag="psi")
                    mm = 0
                    for gT, V in ((gr_t, Vr), (gi_t, Vi)):
                        for it, (ko, kw) in enumerate(KPART):
                            nc.tensor.matmul(
                                ps[:], gT[it][:kw, tt * 128:(tt + 1) * 128], V[it][:],
                                start=(mm == 0), stop=(mm == 9),
                            )
                            mm += 1
                    yf = workp.tile([128, DH], dt.float32, tag="yf")
                    nc.vector.tensor_scalar(yf[:], ps[:], -127.0, 127.0,
                                            mybir.AluOpType.max,
                                            mybir.AluOpType.min)
                    yq_t = workp.tile([128, DH], dt.int8, tag="yqt")
                    nc.vector.tensor_copy(yq_t[:], yf[:])
                    nc.sync.dma_start(
                        yq_d[w * CH + tt * 128: w * CH + (tt + 1) * 128, :], yq_t[:]
                    )
    nc.finalize()
    return nc


# ---------------------------------------------------------------- runner
def _make_runner(nc):
    import jax
    from jax.sharding import Mesh, PartitionSpec
    from jax.experimental.shard_map import shard_map
    import concourse.mybir as mybir
    from concourse.bass2jax import install_neuronx_cc_hook, _bass_exec_p, partition_id_tensor

    install_neuronx_cc_hook()
    in_names, out_names, out_avals, zero_outs = [], [], [], []
    partition_name = nc.partition_id_tensor.name if nc.partition_id_tensor else None
    for alloc in nc.m.functions[0].allocations:
        if not isinstance(alloc, mybir.MemoryLocationSet):
            continue
        name = alloc.memorylocations[0].name
        if alloc.kind == "ExternalInput":
            if name != partition_name:
                in_names.append(name)
        elif alloc.kind == "ExternalOutput":
            out_names.append(name)
            shape = tuple(alloc.tensor_shape)
            dtype = mybir.dt.np(alloc.dtype)
            out_avals.append(jax.core.ShapedArray(shape, dtype))
            zero_outs.append(np.zeros(shape, dtype))
    n_params = len(in_names)
    all_names = in_names + out_names
    if partition_name is not None:
        all_names.append(partition_name)

    def _body(*args):
        operands = list(args)
        if partition_name is not None:
            operands.append(partition_id_tensor())
        outs = _bass_exec_p.bind(
            *operands,
            out_avals=tuple(out_avals),
            in_names=tuple(all_names),
            out_names=tuple(out_names),
            lowering_input_output_aliases=(),
            sim_require_finite=True,
            sim_require_nnan=True,
            nc=nc,
        )
        return tuple(outs)

    devices = jax.devices()[:NCORE]
    mesh = Mesh(np.asarray(devices), ("core",))
    n_outs = len(out_names)
    sharded = jax.jit(
        shard_map(
            _body, mesh=mesh,
            in_specs=(PartitionSpec("core"),) * (n_params + n_outs),
            out_specs=(PartitionSpec("core"),) * n_outs,
            check_rep=False,
        ),
        keep_unused=True,
    )
    return sharded, in_names, out_names, zero_outs, mesh


def _dft_consts():
    t = np.arange(M)
    k = np.arange(KF)
    ang = 2.0 * np.pi / M * np.outer(t, k)
    fc = np.cos(ang)
    fs = -np.sin(ang)
    w_k = np.where((k == 0) | (k == M // 2), 1.0, 2.0) / M
    angi = 2.0 * np.pi / M * np.outer(k, np.arange(CH, M))
    gr = w_k[:, None] * np.cos(angi)
    gi = -w_k[:, None] * np.sin(angi)
    bf = ml_dtypes.bfloat16
    return (fc.astype(bf), fs.astype(bf), gr.astype(bf), gi.astype(bf))


def _init():
    import jax
    from jax.sharding import NamedSharding, PartitionSpec

    nc = _build_nc()
    sharded, in_names, out_names, zero_outs, mesh = _make_runner(nc)
    assert in_names == ["uq", "sab", "kr", "ki", "fc", "fs", "gr", "gi"], in_names
    repl = NamedSharding(mesh, PartitionSpec("core"))
    fc, fs, gr, gi = _dft_consts()
    tile8 = lambda a: jax.device_put(np.tile(a, (NCORE, 1)), repl)
    _S["fgdev"] = (tile8(fc), tile8(fs), tile8(gr), tile8(gi))
    zc = np.zeros((NCORE * L, DH), np.int8)
    _S["zeros"] = jax.device_put(zc, repl)
    _S["sharded"] = sharded
    _S["repl"] = repl
    _S["kcache"] = {}
    try:
        _S["cq"] = _build_cquant()
    except Exception:
        _S["cq"] = None  # numpy fallback
    _S["ready"] = True


# ---------------------------------------------------------------- host math
def _host_precompute(Lr, Li, Cr, Ci, Dp, g, b):
    lam = -np.exp(Lr.astype(np.float64)) + 1j * np.exp(Li.astype(np.float64))
    Ct = (Cr.astype(np.float64) + 1j * Ci.astype(np.float64)) * (np.exp(lam) - 1.0) / lam
    tau = np.arange(KT)
    E = np.exp(lam[None, :] * tau[:, None])  # [KT, N]
    K = (E.real.astype(np.float32) @ Ct.real.T.astype(np.float32)
         - E.imag.astype(np.float32) @ Ct.imag.T.astype(np.float32))  # [KT, D]
    gf = g.astype(np.float32)
    Khat = K * gf[None, :]
    Khat[0] += (Dp * g).astype(np.float32)
    sigma = np.sqrt((Khat.astype(np.float64) ** 2).sum(0))
    s_d = np.maximum(QCLIP * sigma / 127.0, 1e-12).astype(np.float32)
    Kf = np.fft.rfft(Khat / s_d[None, :], n=M, axis=0)
    bf = ml_dtypes.bfloat16
    kr = Kf.real.astype(bf)
    ki = Kf.imag.astype(bf)
    # per-core d-quarter slices, stacked core-major for the sharded input
    krs = np.ascontiguousarray(
        np.concatenate([kr[:, (c % NQ) * DH:(c % NQ + 1) * DH] for c in range(NCORE)], 0))
    kis = np.ascontiguousarray(
        np.concatenate([ki[:, (c % NQ) * DH:(c % NQ + 1) * DH] for c in range(NCORE)], 0))
    if np.any(b):
        # beta rides through the raw conv + residual: conv(beta*1, K) + D*beta
        csK = np.cumsum(K, axis=0)
        off = b.astype(np.float32)[None, :] * (csK + Dp.astype(np.float32)[None, :])
    else:
        off = None
    return krs, kis, s_d, off


_C_SRC = r'''
#include <stdint.h>
#include <math.h>
#define L 4096
#define D 1024
#define NQ 4
#define DH 256
/* Fused LN-stats + raw-x int8 quantization for one batch pair.
   Per row: one cached read computes sum/sumsq + per-quarter max/min, then
   quantizes from L1. Matches the numpy path: q = round(x * 127/amax),
   sab[c][0][l] = amax*rstd/127, sab[c][1][l] = -mu*rstd. */
/* Fused dequant for one batch pair: y[2p+c/4][l][(c%4)*DH+j] = yq[c][l][j] *
   sd[(c%4)*DH+j]. One vectorized int8->f32 cvt+mul pass, strided dst. */
void deq_pair(const int8_t* restrict yq, const float* restrict sd,
              float* restrict y, long p) {
    for (int c = 0; c < 8; c++) {
        long bi = 2 * p + c / NQ;
        int q = c % NQ;
        const float* sq = sd + q * DH;
        for (int l = 0; l < L; l++) {
            const int8_t* src = yq + ((long)c * L + l) * DH;
            float* dst = y + (bi * L + l) * (long)D + q * DH;
            for (int j = 0; j < DH; j++)
                dst[j] = (float)src[j] * sq[j];
        }
    }
}

void quant_pair(const float* restrict xp, int8_t* restrict uq,
                float* restrict sab) {
    for (int bl = 0; bl < 2; bl++) {
        const float* xb = xp + (long)bl * L * D;
        for (int l = 0; l < L; l++) {
            const float* row = xb + (long)l * D;
            double s = 0.0, ss = 0.0;
            float mx[NQ], mn[NQ];
            for (int q = 0; q < NQ; q++) {
                const float* seg = row + q * DH;
                float qmx = seg[0], qmn = seg[0], qs = 0.0f, qss = 0.0f;
                for (int j = 0; j < DH; j++) {
                    float v = seg[j];
                    qs += v; qss += v * v;
                    qmx = v > qmx ? v : qmx;
                    qmn = v < qmn ? v : qmn;
                }
                s += qs; ss += qss;
                mx[q] = qmx; mn[q] = qmn;
            }
            float mu = (float)(s / D);
            float var = (float)(ss / D) - mu * mu;
            var = fmaxf(var, 0.0f);  /* f32 cancellation can go negative on
                                        (near-)constant rows -> sqrt(NaN) */
            float rstd = 1.0f / sqrtf(var + 1e-5f);
            float nmu = -mu * rstd;
            for (int q = 0; q < NQ; q++) {
                float amax = fmaxf(mx[q], -mn[q]);
                amax = fmaxf(amax, 1e-30f);
                float sq = 127.0f / amax;
                const float* seg = row + q * DH;
                int c = NQ * bl + q;
                int8_t* out = uq + ((long)c * L + l) * DH;
                /* round-to-nearest-even via the 1.5*2^23 magic add; the low
                   byte of the float is the int8 (vectorizes, unlike lrintf) */
                for (int j = 0; j < DH; j++) {
                    union { float f; uint32_t u; } t;
                    t.f = seg[j] * sq + 12582912.0f;
                    out[j] = (int8_t)(t.u & 0xffu);
                }
                sab[((long)c * 2 + 0) * L + l] = amax * rstd * (1.0f / 127.0f);
                sab[((long)c * 2 + 1) * L + l] = nmu;
            }
        }
    }
}
'''


def _build_cquant():
    import ctypes, os, subprocess
    h = hashlib.sha1(_C_SRC.encode()).hexdigest()[:16]
    so = f"/tmp/dssq_{h}.so"
    if not os.path.exists(so):
        cf = f"/tmp/dssq_{h}.c"
        with open(cf, "w") as f:
            f.write(_C_SRC)
        subprocess.run(
            ["gcc", "-O3", "-march=native", "-funroll-loops", "-shared",
             "-fPIC", "-o", so + ".tmp", cf],
            check=True, capture_output=True)
        os.replace(so + ".tmp", so)
    lib = ctypes.CDLL(so)
    lib.quant_pair.argtypes = [ctypes.c_void_p] * 3
    lib.quant_pair.restype = None
    lib.deq_pair.argtypes = [ctypes.c_void_p] * 3 + [ctypes.c_long]
    lib.deq_pair.restype = None
    return lib


_MAGIC = np.float32(12582912.0)  # 1.5 * 2^23: adding forces round-to-nearest-even
                                 # into the mantissa; low byte is then the int8.


def _quant_pair(x, p, uq_cc, sab_cc, tf):
    """LN stats + raw-x int8 quantization for batch pair p (batches 2p, 2p+1).

    Core c of the call handles batch 2p + c//NQ, channel quarter c%NQ.
    sab_cc[c] holds [scale_row | offset_row] (2L floats) for core c.
    """
    tb = tf.view(np.int8)[:, ::4]  # low byte of each f32 (little-endian)
    for bl in range(BPC):
        bi = BPC * p + bl
        xb = x[bi]
        mu = xb.mean(1)
        sq = np.einsum('lc,lc->l', xb, xb, optimize=True)
        var = sq * (1.0 / D) - mu * mu
        rstd = 1.0 / np.sqrt(var + LN_EPS)
        nmu = -mu * rstd
        for q in range(NQ):
            c = NQ * bl + q
            xh = xb[:, q * DH:(q + 1) * DH]
            amax = np.maximum(xh.max(1), -xh.min(1))
            np.maximum(amax, 1e-30, out=amax)
            s_q = np.float32(127.0) / amax
            np.multiply(xh, s_q[:, None], out=tf)
            np.add(tf, _MAGIC, out=tf)
            np.copyto(uq_cc[c * L:(c + 1) * L], tb)
            sab_cc[c, 0] = amax * rstd * np.float32(1.0 / 127.0)
            sab_cc[c, 1] = nmu


def kernel(x, Lambda_real, Lambda_imag, C_real, C_imag, param_D, ln_gamma, ln_beta):
    with _LOCK:
        if not _S.get("ready"):
            _init()

    x = np.asarray(x)
    small = [np.asarray(a) for a in (Lambda_real, Lambda_imag, C_real, C_imag,
                                     param_D, ln_gamma, ln_beta)]
    kc = _S["kcache"]
    ent = kc.get("ent")
    if ent is None or not all(
            a.shape == b.shape and np.array_equal(a, b)
            for a, b in zip(small, ent[0])):
        import jax
        krs, kis, s_d, off = _host_precompute(*small)
        kr_dev = jax.device_put(krs, _S["repl"])
        ki_dev = jax.device_put(kis, _S["repl"])
        ent = ([a.copy() for a in small], kr_dev, ki_dev, s_d, off)
        kc["ent"] = ent
    _, kr_dev, ki_dev, s_d, off = ent
    fc_dev, fs_dev, gr_dev, gi_dev = _S["fgdev"]
    sharded, zeros = _S["sharded"], _S["zeros"]

    # Staging buffers are reused across calls (internal only — the returned y
    # is always fresh; previous call's transfers completed before its fetches
    # returned, so overwriting here is safe).
    if "bufs" not in _S:
        _S["bufs"] = (np.empty((L, DH), np.float32),
                      [np.empty((NCORE * L, DH), np.int8) for _ in range(2)],
                      [np.empty((NCORE, 2, L), np.float32) for _ in range(2)])
    tf, uq, sab = _S["bufs"]

    # quantize pair A -> dispatch A (h2d streams while we quantize pair B in
    # the ~65% idle CPU) -> dispatch B; exec A's round trip hides under B's
    # upload, exec B under A's download.
    cq = _S["cq"]
    if cq is not None and not (x.flags.c_contiguous and x.dtype == np.float32):
        x = np.ascontiguousarray(x, np.float32)
    outs = [None, None]
    for p in range(2):
        if cq is not None:
            cq.quant_pair(x.ctypes.data + x.strides[0] * BPC * p,
                          uq[p].ctypes.data, sab[p].ctypes.data)
        else:
            _quant_pair(x, p, uq[p], sab[p], tf)
        outs[p] = sharded(uq[p], sab[p].reshape(-1),
                          kr_dev, ki_dev, fc_dev, fs_dev, gr_dev, gi_dev,
                          zeros)

    # Fetch both outputs from threads so both d2h requests are queued up
    # front; pre-fault y's pages during the wire wait.
    y = np.empty((B, L, D), np.float32)
    yqs = [None, None]

    def _fetch(p):
        yqs[p] = np.asarray(outs[p][0]).reshape(NCORE, L, DH)

    ths = [threading.Thread(target=_fetch, args=(p,)) for p in range(2)]
    for t in ths:
        t.start()
    y.reshape(-1)[::1024] = 0.0  # touch every 4 KB page while the wire streams

    sd_c = np.ascontiguousarray(s_d)
    for p in range(2):
        ths[p].join()
        yq = yqs[p]
        if cq is not None and yq.flags.c_contiguous:
            cq.deq_pair(yq.ctypes.data, sd_c.ctypes.data, y.ctypes.data, p)
            continue
        for c in range(NCORE):
            bi, q = BPC * p + c // NQ, c % NQ
            sd_q = s_d[q * DH:(q + 1) * DH]
            dst = y[bi, :, q * DH:(q + 1) * DH]
            np.multiply(yq[c], sd_q[None, :], out=dst)
    if off is not None:
        y[:, :KT] += off[None]
        y[:, KT:] += off[-1][None, None]
    return y


# revision 25
# speedup vs baseline: 1.1102x; 1.0117x over previous
"""DSS layer (LN -> long causal conv via overlap-save DFT matmuls -> +residual)
on 8 axon-tunneled TRN2 NeuronCores, written in Bass/Tile.

Wall-clock on this setup is dominated by the shared ~45 MB/s axon tunnel
(~85 ms round-trip latency; the CPU is only ~35% busy while the wire streams,
so host numpy CAN hide inside transfer windows, but separate device_put calls
and extra synchronous round trips are expensive). The design minimizes
transferred bytes and keeps the wire busy end-to-end:
  sharding: one program where core = (batch in a pair) x (channel quarter);
        dispatched twice (batches 0-1, then 2-3). Each core owns the full
        sequence for 256 channels -> causal conv needs NO halo rows (total
        upload exactly L*D int8 = 16.8 MB). The split pipelines the tunnel:
        quant of pair B runs in the idle CPU while pair A uploads, exec A's
        round trip hides under pair B's upload stream, dequant of pair A
        hides under pair B's download stream, and exec B hides under pair
        A's download. Both fetches run in threads so their requests are
        queued before any host work.
  host:  per-row mean/var of x (reductions only, no centered temp), per
        (row, quarter) amax of RAW x, int8 quantize q = round(x * 127/amax)
        via the 1.5*2^23 magic-add trick (no rint pass). The LN affine is
        folded into per-row scale a = amax*rstd/127 and offset o = -mu*rstd
        applied on device: u = q*a + o (fused tensor_scalar mul+add). A
        helper thread pre-faults the 64 MB output array during the wire wait.
  device (per core): dequant -> windowed rFFT-as-matmul (shared F, 8 x
        1024-row overlap-save windows, first window zero history), pointwise
        *Kf (gamma / D-residual delta tap / per-channel output scale folded
        in), inverse rFFT-as-matmul (shared G) -> centered int8 quantize.
  download int8, host dequant = single multiply by s_d; beta offset exact
        (off = beta * (cumsum(K) + D) — NOT gamma-scaled: the reference
        convolves raw K against u*gamma + beta).

Transfers ride the jitted shard_map dispatches; constants and the Kf spectrum
are cached on device across calls.
"""
import hashlib
import threading
import numpy as np
import ml_dtypes

B, L, D, N = 4, 4096, 1024, 512
NQ = 4              # channel quarters
DH = D // NQ        # 256 channels per core
CH = 512            # output chunk per window
M = 1024            # DFT window (overlap-save)
KT = 513            # kernel taps kept (<= M - CH + 1): exact for decaying K
KF = M // 2 + 1     # 513 rfft bins
NW = L // CH        # 8 windows per core
NCORE = 8
BPC = 2             # batches per call
LN_EPS = 1e-5
QCLIP = 5.2
KPART = [(0, 128), (128, 128), (256, 128), (384, 128), (512, 1)]

_S = {}
_LOCK = threading.Lock()


# ---------------------------------------------------------------- device kernel
def _build_nc():
    import concourse.bacc as bacc
    import concourse.mybir as mybir
    import concourse.tile as tile

    dt = mybir.dt
    nc = bacc.Bacc("TRN2", target_bir_lowering=False, debug=False, num_devices=NCORE)
    uq_d = nc.dram_tensor("uq", [L, DH], dt.int8, kind="ExternalInput").ap()
    sab_d = nc.dram_tensor("sab", [2 * L], dt.float32, kind="ExternalInput").ap()
    kr_d = nc.dram_tensor("kr", [KF, DH], dt.bfloat16, kind="ExternalInput").ap()
    ki_d = nc.dram_tensor("ki", [KF, DH], dt.bfloat16, kind="ExternalInput").ap()
    fc_d = nc.dram_tensor("fc", [M, KF], dt.bfloat16, kind="ExternalInput").ap()
    fs_d = nc.dram_tensor("fs", [M, KF], dt.bfloat16, kind="ExternalInput").ap()
    gr_d = nc.dram_tensor("gr", [KF, CH], dt.bfloat16, kind="ExternalInput").ap()
    gi_d = nc.dram_tensor("gi", [KF, CH], dt.bfloat16, kind="ExternalInput").ap()
    yq_d = nc.dram_tensor("yq", [L, DH], dt.int8, kind="ExternalOutput").ap()

    nT = L // 128  # 32 row tiles of 128

    with tile.TileContext(nc) as tc:
        with (
            tc.tile_pool(name="const", bufs=1) as constp,
            tc.tile_pool(name="stage", bufs=2) as stagep,
            tc.tile_pool(name="upool", bufs=9) as upool,
            tc.tile_pool(name="uv", bufs=2) as uvp,
            tc.tile_pool(name="work", bufs=2) as workp,
            tc.tile_pool(name="psum", bufs=4, space="PSUM") as psump,
            tc.tile_pool(name="psumi", bufs=2, space="PSUM") as psumip,
        ):
            def widen(dram_ap, rows, cols, tagn):
                st = stagep.tile([rows, cols], dt.bfloat16, tag="stage")
                nc.sync.dma_start(st[:], dram_ap)
                ft = constp.tile([rows, cols], dt.float32, tag=tagn)
                nc.vector.tensor_copy(ft[:], st[:])
                return ft

            fc_t = [widen(fc_d[i * 128:(i + 1) * 128, :], 128, KF, f"fc{i}") for i in range(8)]
            fs_t = [widen(fs_d[i * 128:(i + 1) * 128, :], 128, KF, f"fs{i}") for i in range(8)]
            gr_t = [widen(gr_d[o:o + w, :], w, CH, f"gr{i}") for i, (o, w) in enumerate(KPART)]
            gi_t = [widen(gi_d[o:o + w, :], w, CH, f"gi{i}") for i, (o, w) in enumerate(KPART)]

            # Kf stays bf16 in SBUF (read by DVE pointwise; halves footprint)
            def load_bf(dram_ap, rows, cols, tagn):
                t = constp.tile([rows, cols], dt.bfloat16, tag=tagn)
                nc.sync.dma_start(t[:], dram_ap)
                return t

            kr_t = [load_bf(kr_d[o:o + w, :], w, DH, f"kr{i}") for i, (o, w) in enumerate(KPART)]
            ki_t = [load_bf(ki_d[o:o + w, :], w, DH, f"ki{i}") for i, (o, w) in enumerate(KPART)]

            sa_raw = constp.tile([128, nT], dt.float32, tag="sa_raw")
            nc.sync.dma_start(sa_raw[:], sab_d[0:L].rearrange("(n p) -> p n", p=128))
            so_raw = constp.tile([128, nT], dt.float32, tag="so_raw")
            nc.sync.dma_start(so_raw[:], sab_d[L:2 * L].rearrange("(n p) -> p n", p=128))
            # staged via same-engine copy so dequant TensorScalarPtr needs no waits
            sa_sb = constp.tile([128, nT], dt.float32, tag="sa_sb")
            nc.vector.tensor_copy(sa_sb[:], sa_raw[:])
            so_sb = constp.tile([128, nT], dt.float32, tag="so_sb")
            nc.vector.tensor_copy(so_sb[:], so_raw[:])

            mult = mybir.AluOpType.mult
            add = mybir.AluOpType.add

            for w in range(NW):
                # window rows [w*512-512, w*512+512); slot si holds global tile
                # gt = w*4 - 4 + si; for w == 0 slots 0-3 are zero history and
                # their matmuls are skipped entirely.
                s0 = 4 if w == 0 else 0
                u_t = [None] * 8
                for si in range(s0, 8):
                    gt = w * 4 - 4 + si
                    stq = stagep.tile([128, DH], dt.int8, tag="uqstage")
                    nc.sync.dma_start(stq[:], uq_d[gt * 128:(gt + 1) * 128, :])
                    uf = upool.tile([128, DH], dt.float32, tag="u")
                    nc.vector.tensor_scalar(
                        uf[:], stq[:], sa_sb[:, gt:gt + 1], so_sb[:, gt:gt + 1],
                        mult, add,
                    )
                    u_t[si] = uf
                Vr, Vi = [], []
                for it, (ko, kw) in enumerate(KPART):
                    sb_ri = []
                    for nm, fT in (("ur", fc_t), ("ui", fs_t)):
                        ps = psump.tile([kw, DH], dt.float32, tag="psf")
                        for si in range(s0, 8):
                            nc.tensor.matmul(
                                ps[:], fT[si][:, ko:ko + kw], u_t[si][:],
                                start=(si == s0), stop=(si == 7),
                            )
                        sb = uvp.tile([kw, DH], dt.float32, tag=nm)
                        nc.scalar.copy(sb[:], ps[:])
                        sb_ri.append(sb)
                    ur, ui = sb_ri
                    krs, kis = kr_t[it][:kw, :], ki_t[it][:kw, :]
                    t1 = workp.tile([kw, DH], dt.float32, tag="t1")
                    t2 = workp.tile([kw, DH], dt.float32, tag="t2")
                    nc.vector.tensor_mul(t1[:], ur[:], krs)
                    nc.vector.tensor_mul(t2[:], ui[:], kis)
                    vr = uvp.tile([kw, DH], dt.float32, tag=f"vr{it}")
                    nc.vector.tensor_sub(vr[:], t1[:], t2[:])
                    t3 = workp.tile([kw, DH], dt.float32, tag="t3")
                    t4 = workp.tile([kw, DH], dt.float32, tag="t4")
                    nc.vector.tensor_mul(t3[:], ur[:], kis)
                    nc.vector.tensor_mul(t4[:], ui[:], krs)
                    vi = uvp.tile([kw, DH], dt.float32, tag=f"vi{it}")
                    nc.vector.tensor_add(vi[:], t3[:], t4[:])
                    Vr.append(vr)
                    Vi.append(vi)
                for tt in range(4):
                    ps = psumip.tile([128, DH], dt.float32, tag="psi")
                    mm = 0
                    for gT, V in ((gr_t, Vr), (gi_t, Vi)):
                        for it, (ko, kw) in enumerate(KPART):
                            nc.tensor.matmul(
                                ps[:], gT[it][:kw, tt * 128:(tt + 1) * 128], V[it][:],
                                start=(mm == 0), stop=(mm == 9),
                            )
                            mm += 1
                    yf = workp.tile([128, DH], dt.float32, tag="yf")
                    nc.vector.tensor_scalar(yf[:], ps[:], -127.0, 127.0,
                                            mybir.AluOpType.max,
                                            mybir.AluOpType.min)
                    yq_t = workp.tile([128, DH], dt.int8, tag="yqt")
                    nc.vector.tensor_copy(yq_t[:], yf[:])
                    nc.sync.dma_start(
                        yq_d[w * CH + tt * 128: w * CH + (tt + 1) * 128, :], yq_t[:]
                    )
    nc.finalize()
    return nc


# ---------------------------------------------------------------- runner
def _make_runner(nc):
    import jax
    from jax.sharding import Mesh, PartitionSpec
    from jax.experimental.shard_map import shard_map
    import concourse.mybir as mybir
    from concourse.bass2jax import install_neuronx_cc_hook, _bass_exec_p, partition_id_tensor

    install_neuronx_cc_hook()
    in_names, out_names, out_avals, zero_outs = [], [], [], []
    partition_name = nc.partition_id_tensor.name if nc.partition_id_tensor else None
    for alloc in nc.m.functions[0].allocations:
        if not isinstance(alloc, mybir.MemoryLocationSet):
            continue
        name = alloc.memorylocations[0].name
        if alloc.kind == "ExternalInput":
            if name != partition_name:
                in_names.append(name)
        elif alloc.kind == "ExternalOutput":
            out_names.append(name)
            shape = tuple(alloc.tensor_shape)
            dtype = mybir.dt.np(alloc.dtype)
            out_avals.append(jax.core.ShapedArray(shape, dtype))
            zero_outs.append(np.zeros(shape, dtype))
    n_params = len(in_names)
    all_names = in_names + out_names
    if partition_name is not None:
        all_names.append(partition_name)

    def _body(*args):
        operands = list(args)
        if partition_name is not None:
            operands.append(partition_id_tensor())
        outs = _bass_exec_p.bind(
            *operands,
            out_avals=tuple(out_avals),
            in_names=tuple(all_names),
            out_names=tuple(out_names),
            lowering_input_output_aliases=(),
            sim_require_finite=True,
            sim_require_nnan=True,
            nc=nc,
        )
        return tuple(outs)

    devices = jax.devices()[:NCORE]
    mesh = Mesh(np.asarray(devices), ("core",))
    n_outs = len(out_names)
    sharded = jax.jit(
        shard_map(
            _body, mesh=mesh,
            in_specs=(PartitionSpec("core"),) * (n_params + n_outs),
            out_specs=(PartitionSpec("core"),) * n_outs,
            check_rep=False,
        ),
        keep_unused=True,
    )
    return sharded, in_names, out_names, zero_outs, mesh


def _dft_consts():
    t = np.arange(M)
    k = np.arange(KF)
    ang = 2.0 * np.pi / M * np.outer(t, k)
    fc = np.cos(ang)
    fs = -np.sin(ang)
    w_k = np.where((k == 0) | (k == M // 2), 1.0, 2.0) / M
    angi = 2.0 * np.pi / M * np.outer(k, np.arange(CH, M))
    gr = w_k[:, None] * np.cos(angi)
    gi = -w_k[:, None] * np.sin(angi)
    bf = ml_dtypes.bfloat16
    return (fc.astype(bf), fs.astype(bf), gr.astype(bf), gi.astype(bf))


def _init():
    import jax
    from jax.sharding import NamedSharding, PartitionSpec

    nc = _build_nc()
    sharded, in_names, out_names, zero_outs, mesh = _make_runner(nc)
    assert in_names == ["uq", "sab", "kr", "ki", "fc", "fs", "gr", "gi"], in_names
    repl = NamedSharding(mesh, PartitionSpec("core"))
    fc, fs, gr, gi = _dft_consts()
    tile8 = lambda a: jax.device_put(np.tile(a, (NCORE, 1)), repl)
    _S["fgdev"] = (tile8(fc), tile8(fs), tile8(gr), tile8(gi))
    zc = np.zeros((NCORE * L, DH), np.int8)
    _S["zeros"] = jax.device_put(zc, repl)
    _S["sharded"] = sharded
    _S["repl"] = repl
    _S["kcache"] = {}
    try:
        _S["cq"] = _build_cquant()
    except Exception:
        _S["cq"] = None  # numpy fallback
    _S["ready"] = True


# ---------------------------------------------------------------- host math
def _host_precompute(Lr, Li, Cr, Ci, Dp, g, b):
    lam = -np.exp(Lr.astype(np.float64)) + 1j * np.exp(Li.astype(np.float64))
    Ct = (Cr.astype(np.float64) + 1j * Ci.astype(np.float64)) * (np.exp(lam) - 1.0) / lam
    tau = np.arange(KT)
    E = np.exp(lam[None, :] * tau[:, None])  # [KT, N]
    K = (E.real.astype(np.float32) @ Ct.real.T.astype(np.float32)
         - E.imag.astype(np.float32) @ Ct.imag.T.astype(np.float32))  # [KT, D]
    gf = g.astype(np.float32)
    Khat = K * gf[None, :]
    Khat[0] += (Dp * g).astype(np.float32)
    sigma = np.sqrt((Khat.astype(np.float64) ** 2).sum(0))
    s_d = np.maximum(QCLIP * sigma / 127.0, 1e-12).astype(np.float32)
    Kf = np.fft.rfft(Khat / s_d[None, :], n=M, axis=0)
    bf = ml_dtypes.bfloat16
    kr = Kf.real.astype(bf)
    ki = Kf.imag.astype(bf)
    # per-core d-quarter slices, stacked core-major for the sharded input
    krs = np.ascontiguousarray(
        np.concatenate([kr[:, (c % NQ) * DH:(c % NQ + 1) * DH] for c in range(NCORE)], 0))
    kis = np.ascontiguousarray(
        np.concatenate([ki[:, (c % NQ) * DH:(c % NQ + 1) * DH] for c in range(NCORE)], 0))
    if np.any(b):
        # beta rides through the raw conv + residual: conv(beta*1, K) + D*beta
        csK = np.cumsum(K, axis=0)
        off = b.astype(np.float32)[None, :] * (csK + Dp.astype(np.float32)[None, :])
    else:
        off = None
    return krs, kis, s_d, off


_C_SRC = r'''
#include <stdint.h>
#include <math.h>
#define L 4096
#define D 1024
#define NQ 4
#define DH 256
/* Fused LN-stats + raw-x int8 quantization for one batch pair.
   Per row: one cached read computes sum/sumsq + per-quarter max/min, then
   quantizes from L1. Matches the numpy path: q = round(x * 127/amax),
   sab[c][0][l] = amax*rstd/127, sab[c][1][l] = -mu*rstd. */
/* Fused dequant for one batch pair: y[2p+c/4][l][(c%4)*DH+j] = yq[c][l][j] *
   sd[(c%4)*DH+j]. One vectorized int8->f32 cvt+mul pass, strided dst. */
void deq_pair(const int8_t* restrict yq, const float* restrict sd,
              float* restrict y, long p) {
    for (int c = 0; c < 8; c++) {
        long bi = 2 * p + c / NQ;
        int q = c % NQ;
        const float* sq = sd + q * DH;
        for (int l = 0; l < L; l++) {
            const int8_t* src = yq + ((long)c * L + l) * DH;
            float* dst = y + (bi * L + l) * (long)D + q * DH;
            for (int j = 0; j < DH; j++)
                dst[j] = (float)src[j] * sq[j];
        }
    }
}

void quant_pair(const float* restrict xp, int8_t* restrict uq,
                float* restrict sab) {
    for (int bl = 0; bl < 2; bl++) {
        const float* xb = xp + (long)bl * L * D;
        for (int l = 0; l < L; l++) {
            const float* row = xb + (long)l * D;
            double s = 0.0, ss = 0.0;
            float mx[NQ], mn[NQ];
            for (int q = 0; q < NQ; q++) {
                const float* seg = row + q * DH;
                float qmx = seg[0], qmn = seg[0], qs = 0.0f, qss = 0.0f;
                for (int j = 0; j < DH; j++) {
                    float v = seg[j];
                    qs += v; qss += v * v;
                    qmx = v > qmx ? v : qmx;
                    qmn = v < qmn ? v : qmn;
                }
                s += qs; ss += qss;
                mx[q] = qmx; mn[q] = qmn;
            }
            float mu = (float)(s / D);
            float var = (float)(ss / D) - mu * mu;
            var = fmaxf(var, 0.0f);  /* f32 cancellation can go negative on
                                        (near-)constant rows -> sqrt(NaN) */
            float rstd = 1.0f / sqrtf(var + 1e-5f);
            float nmu = -mu * rstd;
            for (int q = 0; q < NQ; q++) {
                float amax = fmaxf(mx[q], -mn[q]);
                amax = fmaxf(amax, 1e-30f);
                float sq = 127.0f / amax;
                const float* seg = row + q * DH;
                int c = NQ * bl + q;
                int8_t* out = uq + ((long)c * L + l) * DH;
                /* round-to-nearest-even via the 1.5*2^23 magic add; the low
                   byte of the float is the int8 (vectorizes, unlike lrintf) */
                for (int j = 0; j < DH; j++) {
                    union { float f; uint32_t u; } t;
                    t.f = seg[j] * sq + 12582912.0f;
                    out[j] = (int8_t)(t.u & 0xffu);
                }
                sab[((long)c * 2 + 0) * L + l] = amax * rstd * (1.0f / 127.0f);
                sab[((long)c * 2 + 1) * L + l] = nmu;
            }
        }
    }
}
'''


def _build_cquant():
    import ctypes, os, subprocess
    # -Ofast: the stats loops are float reductions, which gcc only vectorizes
    # with reassociation allowed. The var>=0 clamp guards the one spot where
    # summation order matters; inputs are finite by construction.
    flags = ["-Ofast", "-march=native", "-funroll-loops", "-shared", "-fPIC"]
    h = hashlib.sha1((_C_SRC + " ".join(flags)).encode()).hexdigest()[:16]
    so = f"/tmp/dssq_{h}.so"
    if not os.path.exists(so):
        cf = f"/tmp/dssq_{h}.c"
        with open(cf, "w") as f:
            f.write(_C_SRC)
        subprocess.run(
            ["gcc", *flags, "-o", so + ".tmp", cf],
            check=True, capture_output=True)
        os.replace(so + ".tmp", so)
    lib = ctypes.CDLL(so)
    lib.quant_pair.argtypes = [ctypes.c_void_p] * 3
    lib.quant_pair.restype = None
    lib.deq_pair.argtypes = [ctypes.c_void_p] * 3 + [ctypes.c_long]
    lib.deq_pair.restype = None
    return lib


_MAGIC = np.float32(12582912.0)  # 1.5 * 2^23: adding forces round-to-nearest-even
                                 # into the mantissa; low byte is then the int8.


def _quant_pair(x, p, uq_cc, sab_cc, tf):
    """LN stats + raw-x int8 quantization for batch pair p (batches 2p, 2p+1).

    Core c of the call handles batch 2p + c//NQ, channel quarter c%NQ.
    sab_cc[c] holds [scale_row | offset_row] (2L floats) for core c.
    """
    tb = tf.view(np.int8)[:, ::4]  # low byte of each f32 (little-endian)
    for bl in range(BPC):
        bi = BPC * p + bl
        xb = x[bi]
        mu = xb.mean(1)
        sq = np.einsum('lc,lc->l', xb, xb, optimize=True)
        var = sq * (1.0 / D) - mu * mu
        rstd = 1.0 / np.sqrt(var + LN_EPS)
        nmu = -mu * rstd
        for q in range(NQ):
            c = NQ * bl + q
            xh = xb[:, q * DH:(q + 1) * DH]
            amax = np.maximum(xh.max(1), -xh.min(1))
            np.maximum(amax, 1e-30, out=amax)
            s_q = np.float32(127.0) / amax
            np.multiply(xh, s_q[:, None], out=tf)
            np.add(tf, _MAGIC, out=tf)
            np.copyto(uq_cc[c * L:(c + 1) * L], tb)
            sab_cc[c, 0] = amax * rstd * np.float32(1.0 / 127.0)
            sab_cc[c, 1] = nmu


def kernel(x, Lambda_real, Lambda_imag, C_real, C_imag, param_D, ln_gamma, ln_beta):
    with _LOCK:
        if not _S.get("ready"):
            _init()

    x = np.asarray(x)
    small = [np.asarray(a) for a in (Lambda_real, Lambda_imag, C_real, C_imag,
                                     param_D, ln_gamma, ln_beta)]
    kc = _S["kcache"]
    ent = kc.get("ent")
    if ent is None or not all(
            a.shape == b.shape and np.array_equal(a, b)
            for a, b in zip(small, ent[0])):
        import jax
        krs, kis, s_d, off = _host_precompute(*small)
        kr_dev = jax.device_put(krs, _S["repl"])
        ki_dev = jax.device_put(kis, _S["repl"])
        ent = ([a.copy() for a in small], kr_dev, ki_dev, s_d, off)
        kc["ent"] = ent
    _, kr_dev, ki_dev, s_d, off = ent
    fc_dev, fs_dev, gr_dev, gi_dev = _S["fgdev"]
    sharded, zeros = _S["sharded"], _S["zeros"]

    # Staging buffers are reused across calls (internal only — the returned y
    # is always fresh; previous call's transfers completed before its fetches
    # returned, so overwriting here is safe).
    if "bufs" not in _S:
        _S["bufs"] = (np.empty((L, DH), np.float32),
                      [np.empty((NCORE * L, DH), np.int8) for _ in range(2)],
                      [np.empty((NCORE, 2, L), np.float32) for _ in range(2)])
    tf, uq, sab = _S["bufs"]

    # quantize pair A -> dispatch A (h2d streams while we quantize pair B in
    # the ~65% idle CPU) -> dispatch B; exec A's round trip hides under B's
    # upload, exec B under A's download.
    cq = _S["cq"]
    if cq is not None and not (x.flags.c_contiguous and x.dtype == np.float32):
        x = np.ascontiguousarray(x, np.float32)
    outs = [None, None]
    for p in range(2):
        if cq is not None:
            cq.quant_pair(x.ctypes.data + x.strides[0] * BPC * p,
                          uq[p].ctypes.data, sab[p].ctypes.data)
        else:
            _quant_pair(x, p, uq[p], sab[p], tf)
        outs[p] = sharded(uq[p], sab[p].reshape(-1),
                          kr_dev, ki_dev, fc_dev, fs_dev, gr_dev, gi_dev,
                          zeros)

    # Fetch both outputs from threads so both d2h requests are queued up
    # front; pre-fault y's pages during the wire wait.
    y = np.empty((B, L, D), np.float32)
    yqs = [None, None]

    def _fetch(p):
        yqs[p] = np.asarray(outs[p][0]).reshape(NCORE, L, DH)

    ths = [threading.Thread(target=_fetch, args=(p,)) for p in range(2)]
    for t in ths:
        t.start()
    y.reshape(-1)[::1024] = 0.0  # touch every 4 KB page while the wire streams

    sd_c = np.ascontiguousarray(s_d)
    for p in range(2):
        ths[p].join()
        yq = yqs[p]
        if cq is not None and yq.flags.c_contiguous:
            cq.deq_pair(yq.ctypes.data, sd_c.ctypes.data, y.ctypes.data, p)
            continue
        for c in range(NCORE):
            bi, q = BPC * p + c // NQ, c % NQ
            sd_q = s_d[q * DH:(q + 1) * DH]
            dst = y[bi, :, q * DH:(q + 1) * DH]
            np.multiply(yq[c], sd_q[None, :], out=dst)
    if off is not None:
        y[:, :KT] += off[None]
        y[:, KT:] += off[-1][None, None]
    return y
